# revision 8
# baseline (speedup 1.0000x reference)
"""Trainium2 Bass kernel for nn_Architecture_11879879540882 (AKT-style
monotonic sparse attention), data-parallel over batch on 8 NeuronCores.

Triangular-domain redesign of the dense baseline (~1.24x faster):
 - All per-element phases (E=exp(s), scan, bigA, sqrt, exp, z, Ez) run only on
   the lower-triangular chunk domain cols<=(c+1)*128 for row-chunk c (62.5%).
 - Blocks 1/2: scores are symmetric (K==Q incl bias/scale), so the unmasked
   upper region of Ez equals exp(s) = E of the mirrored lower chunk; the AV
   lhsT chunks for jc>c are direct slices of E(row-chunk jc) - no transpose,
   no extra ACT work, no extra exp.
 - Block 3: scores are broadcast rows (query is position-independent). The
   z-multiply runs in the transposed domain where the score row becomes a
   per-partition column (tensor_scalar fused with the PSUM->SBUF copy), and
   the upper-region AV contribution collapses to rank-1 row vectors
   w = E1d.V added via ones-broadcast matmuls.
 - sqrt/exp/z/Ez ping-pong between bigA and a per-c partner tile P{c}
   (bufs=1, b-granular table windows). In-place activations are AVOIDED:
   they run ~40% slower on HW (same-address read+write).
 - Reverse scans write directly into the P{c} tiles (no separate Rv
   buffers); the chunk's final bigA column is provably zero (suffix beyond
   the diagonal) and is memset after the fact.
 - LayerNorm rstd = Sqrt(1/(var+eps)) is batched into the NEXT block's Sqrt
   table window (no extra table loads for blocks 1/2).
 - Matmul PSUM outputs are kept inside single 2KB banks (ho-stride padded
   to 512); bank-crossing matmuls silently corrupt PSUM on HW.
 - GPSIMD (Pool) cannot touch PSUM and supports only memset/copy/tt/ts;
   it handles memsets only. PSUM->SBUF copies round-robin DVE:ACT 2:1
   (Act.Copy lives in every activation table set).

Validated: CoreSim end-to-end vs the jax reference (rel err 1.2e-3) and on
hardware (fresh-device first run, rel err 1.18e-3). HW time 592 us vs
733 us baseline.
"""
import sys
import numpy as np

for _p in ('/opt/trn_rl_repo',):
    if _p not in sys.path:
        sys.path.append(_p)

import ml_dtypes
import concourse.bass as bass
import concourse.bacc as bacc
import concourse.tile as tile
import concourse.mybir as mybir
from concourse.bass_utils import run_bass_kernel_spmd

F32 = mybir.dt.float32
BF16 = mybir.dt.bfloat16
Alu = mybir.AluOpType
Act = mybir.ActivationFunctionType
NPBF = ml_dtypes.bfloat16

B, S, D, H, DK = 16, 512, 256, 8, 32
NCORES = 8
BL = B // NCORES          # local batches per core = 2
PC = BL * 4               # 128-row position chunks per core = 8
LN_EPS = 1e-5

REV = (slice(None), slice(None, None, -1))


def _softplus(x):
    return np.logaddexp(0.0, x)


def _host_prep(inp):
    """Parameter preprocessing on host. Returns (consts dict, g2 dict)."""
    p = {k: np.asarray(v, np.float32) for k, v in inp.items()}
    c = {}
    s4 = np.float32(DK ** -0.25)
    bf = lambda x: np.ascontiguousarray(np.asarray(x, np.float32)).astype(NPBF)
    colpack = lambda b: np.ascontiguousarray(
        np.asarray(b, np.float32).reshape(2, 128).T).astype(np.float32)

    for blk in ('b1', 'b2'):
        c[blk + '_wq'] = bf(p[blk + '_qw'] * s4)
        c[blk + '_qbr'] = bf((p[blk + '_qb'] * s4)[None, :])
        c[blk + '_wv'] = bf(p[blk + '_vw'])
        c[blk + '_vbr'] = bf(p[blk + '_vb'][None, :])
        c[blk + '_wo'] = bf(p[blk + '_ow'])
        c[blk + '_obr'] = bf(p[blk + '_ob'][None, :])
    know = p['know'][0, 0]
    q03 = ((know @ p['b3_qw'] + p['b3_qb']) / np.sqrt(DK)).reshape(H, DK)
    Q03 = np.zeros((D, H), np.float32)
    for h in range(H):
        Q03[h * DK:(h + 1) * DK, h] = q03[h]
    c['q03'] = bf(Q03)
    g1, be1 = p['b1_lng'], p['b1_lnb']
    c['b3_wk'] = bf(p['b3_kw'] * g1[:, None])
    c['b3_kbT'] = colpack(p['b3_kb'] + be1 @ p['b3_kw'])
    g2_, be2 = p['b2_lng'], p['b2_lnb']
    c['b3_wv'] = bf(p['b3_vw'] * g2_[:, None])
    c['b3_vbr'] = bf((p['b3_vb'] + be2 @ p['b3_vw'])[None, :])
    c['b3_wo'] = bf(p['b3_ow'])
    c['b3_obr'] = bf((p['b3_ob'] + know)[None, :])
    g3, be3 = p['b3_lng'], p['b3_lnb']
    lvw = np.zeros((H, DK, D), np.float32)
    lvb = np.zeros((H, D), np.float32)
    for h in range(H):
        sl = slice(h * DK, (h + 1) * DK)
        lvw[h] = p['lv_w'] * g3[sl][:, None]
        lvb[h] = p['lv_b'] + be3[sl] @ p['lv_w']
    c['lvw'] = bf(lvw)                            # -> lvw__h [32,256]
    c['lvbr'] = bf(lvb.reshape(1, H * D))         # [1, 2048]
    know_r = know.reshape(H, DK)
    keyh = 1.0 / (1.0 + np.exp(-(know_r @ p['lk_w'] + p['lk_b'])))
    c['keyhT'] = bf(keyh.T)                       # [D, H]

    # padded inclusive mask: mpad[ic][p, j] = (j <= i_p), j in [0, 512];
    # strict mask is the shifted view mpad[:, 1:513].
    i = np.arange(S + 1, dtype=np.int64)
    mpad = np.zeros((4, 128, S + 1), np.float32)
    pos = np.zeros((4, 128, S), np.float32)
    for ic in range(4):
        ii = np.arange(ic * 128, (ic + 1) * 128, dtype=np.int64)[:, None]
        mpad[ic] = (i[None, :] <= ii)
        pos[ic] = np.abs(ii - i[None, :S])
    for blk in ('b1', 'b2', 'b3'):
        g2v = (_softplus(p[blk + '_gam'][:, 0, 0]) ** 2).astype(np.float32)
        c['g2b_' + blk] = np.ascontiguousarray(
            np.broadcast_to(g2v[None, :], (128, H))).astype(np.float32)
    c['ident'] = bf(np.eye(128))

    flat = {}
    for name, a in c.items():
        if a.ndim == 2 and a.shape[0] > 128:
            for kc in range(a.shape[0] // 128):
                flat[f"{name}__{kc}"] = np.ascontiguousarray(
                    a[kc * 128:(kc + 1) * 128])
        elif a.ndim == 3:
            for kc in range(a.shape[0]):
                flat[f"{name}__{kc}"] = np.ascontiguousarray(a[kc])
        else:
            flat[name] = a
    # triangular-truncated masks / positions (chunk c needs cols <= (c+1)*128)
    for ic in range(4):
        Lc = (ic + 1) * 128
        flat[f"mpad__{ic}"] = np.ascontiguousarray(bf(mpad)[ic][:, :Lc + 1])
        flat[f"posm__{ic}"] = np.ascontiguousarray(bf(pos)[ic][:, :Lc])
    g2 = {blk: [float(v) for v in
                (_softplus(p[blk + '_gam'][:, 0, 0]) ** 2)]
          for blk in ('b1', 'b2', 'b3')}
    return flat, g2


_NPDT = {np.dtype(np.float32): F32, np.dtype(NPBF): BF16}


def _L(c):
    return (c + 1) * 128


def _build(consts, g2, reps=1):
    """Builds the per-core Bass graph (BL local batches)."""
    nc = bacc.Bacc("TRN2", target_bir_lowering=False, debug=False)

    x1d = nc.dram_tensor("x1", [BL, S, D], F32, kind="ExternalInput")
    x2d = nc.dram_tensor("x2", [BL, S, D], F32, kind="ExternalInput")
    outd = nc.dram_tensor("out", [BL, S, D], F32, kind="ExternalOutput")
    cd = {name: nc.dram_tensor(name, list(a.shape), _NPDT[a.dtype],
                               kind="ExternalInput")
          for name, a in consts.items()}

    from contextlib import ExitStack
    with tile.TileContext(nc) as tc, ExitStack() as _ps:
        sb = _ps.enter_context(tc.tile_pool(name="const", bufs=1))
        work = _ps.enter_context(tc.tile_pool(name="work", bufs=1))
        sm = _ps.enter_context(tc.tile_pool(name="sm", bufs=4))
        p1 = _ps.enter_context(tc.tile_pool(name="p1", bufs=2, space="PSUM"))
        pT = _ps.enter_context(tc.tile_pool(name="pT", bufs=2, space="PSUM"))
        pv = _ps.enter_context(tc.tile_pool(name="pv", bufs=2, space="PSUM"))

        # ---------- constants ----------
        def _prio(name):
            for i, k in enumerate(('ident', 'b1_', 'mpad', 'posm', 'g2b_b1',
                                   'b2_', 'g2b_b2', 'b3_', 'q03', 'g2b_b3',
                                   'lv', 'key')):
                if name.startswith(k):
                    return i
            return 99
        C = {}
        for name in sorted(cd, key=_prio):
            ap = cd[name]
            t = sb.tile(list(ap.shape), ap.dtype, name="c_" + name)
            nc.sync.dma_start(t[:], ap[:])
            C[name] = t
        ones1 = sb.tile([1, 512], BF16, name="ones1")
        nc.vector.memset(ones1[:], 1.0)
        ident = C['ident']

        # round-robin PSUM->SBUF copies between DVE and ACT (GPSIMD cannot
        # access PSUM on TRN2; Act.Copy lives in every table set).
        _rr = [0]

        def rr_copy(dst, src):
            _rr[0] += 1
            if _rr[0] % 3 != 2:
                nc.vector.tensor_copy(dst, src)
            else:
                nc.scalar.activation(dst, src, Act.Copy)

        def transpose128(dst, src):
            """dst[128,128] SBUF bf16 = src.T via PE + Pool/DVE copy."""
            pt = pT.tile([128, 512], BF16, tag="pT", name="pt_t")
            nc.tensor.transpose(pt[:, 0:128], src, ident[:])
            rr_copy(dst, pt[:, 0:128])

        for _rep in range(reps):
          # ---------- input prep: transposed bf16 copies of x1/x2 ----------
          xT = {}
          for xi, xd in ((1, x1d), (2, x2d)):
              for dc in range(2):
                  xT[(xi, dc)] = work.tile([128, BL * S], BF16, tag="xfrm",
                                           bufs=4, name=f"xT{xi}_{dc}")
              for pc in range(PC):
                  b, ic = divmod(pc, 4)
                  t = sm.tile([128, D], F32, tag="xin", name="xin", bufs=2)
                  nc.sync.dma_start(t[:], xd[b, ic * 128:(ic + 1) * 128, :])
                  tb = sm.tile([128, D], BF16, tag="xbf", name="xbf_t", bufs=2)
                  nc.vector.tensor_copy(tb[:], t[:])
                  for dc in range(2):
                      transpose128(xT[(xi, dc)][:, pc * 128:(pc + 1) * 128],
                                   tb[:, dc * 128:(dc + 1) * 128])

          # -------- alpha = softmax_h(x1 . keyh) --------
          alphas = []
          for pc in range(PC):
              psb = pv.tile([128, 264], F32, tag="pv", name="psb")
              for kc in range(2):
                  nc.tensor.matmul(psb[:, 0:H],
                                   xT[(1, kc)][:, pc * 128:(pc + 1) * 128],
                                   C[f"keyhT__{kc}"][:],
                                   start=(kc == 0), stop=(kc == 1))
              ea = sm.tile([128, H], F32, tag="ea", name="ea")
              rsa = sm.tile([128, 1], F32, tag="rsa", name="rsa")
              nc.scalar.activation(ea[:], psb[:, 0:H], Act.Exp,
                                   accum_out=rsa[:])
              ira = sm.tile([128, 1], F32, tag="ira", name="ira")
              nc.vector.reciprocal(ira[:], rsa[:])
              al = sm.tile([128, H], F32, tag="alpha", name="alpha", bufs=8)
              nc.vector.tensor_scalar(al[:], ea[:], ira[:], None,
                                      op0=Alu.mult)
              alphas.append(al)

          # ---------- projections ----------
          def projQ(xTloc, wname, brname, out_name):
              """Head-packed transposed projection QTp [32, H*BL*S]."""
              QTp = work.tile([32, H * BL * S], BF16, name=out_name,
                              tag="QTp", bufs=1)
              qbr = C[brname]
              for h in range(H):
                  for half in range(BL):
                      ps = p1.tile([32, 512], F32, tag="p1", name="projQ_ps")
                      for kc in range(2):
                          nc.tensor.matmul(
                              ps[:],
                              C[f"{wname}__{kc}"][:, h * 32:(h + 1) * 32],
                              xTloc[kc][:, half * 512:(half + 1) * 512],
                              start=(kc == 0), stop=False)
                      nc.tensor.matmul(ps[:], qbr[0:1, h * 32:(h + 1) * 32],
                                       ones1[0:1, :], start=False, stop=True)
                      rr_copy(QTp[:, h * 1024 + half * 512:
                                  h * 1024 + (half + 1) * 512], ps[:])
              return QTp

          def projT(xTloc, wname, bTname, out_name):
              """Chunk-transposed projection out[dc][128, BL*S] (for K3T)."""
              out = [work.tile([128, BL * S], BF16, tag="xfrm", bufs=4,
                               name=f"{out_name}_{dc}") for dc in range(2)]
              bT = C[bTname]
              for dc in range(2):
                  for hh in range(BL):
                      ps = p1.tile([128, 1024], F32, tag="p1", name="projT_ps")
                      for kc in range(2):
                          nc.tensor.matmul(
                              ps[:, 0:512],
                              C[f"{wname}__{kc}"][:, dc * 128:(dc + 1) * 128],
                              xTloc[kc][:, hh * 512:(hh + 1) * 512],
                              start=(kc == 0), stop=(kc == 1))
                      nc.scalar.activation(out[dc][:, hh * 512:(hh + 1) * 512],
                                           ps[:, 0:512], Act.Identity,
                                           bias=bT[:, dc:dc + 1], scale=1.0)
              return out

          def projN(xTloc, wname, brname, out_name):
              """Natural projection, head-packed with a ones column:
              out[pc] [128, 8*33]: head h = cols [33h, 33h+32), col 33h+32=1."""
              out = [work.tile([128, H * 33], BF16, tag="Vt", bufs=8,
                               name=f"{out_name}_{pc}") for pc in range(PC)]
              br = C[brname]
              for pc in range(PC):
                  ps = pv.tile([128, 264], F32, tag="pv", name="projN_ps")
                  for kc in range(2):
                      nc.tensor.matmul(ps[:, 0:256],
                                       xTloc[kc][:, pc * 128:(pc + 1) * 128],
                                       C[f"{wname}__{kc}"],
                                       start=(kc == 0), stop=False)
                  nc.tensor.matmul(ps[:, 0:256], ones1[0:1, 0:128], br[:],
                                   start=False, stop=True)
                  ov = out[pc].rearrange("p (h c) -> p h c", c=33)
                  nc.scalar.activation(ov[:, :, 0:32],
                                       ps[:, 0:256].rearrange(
                                           "p (h c) -> p h c", c=32),
                                       Act.Copy)
                  nc.gpsimd.memset(ov[:, :, 32:33], 1.0)
              return out

          # ---------- LN pending flush ----------
          def flush_ln(pend, use_rsqrt=False):
              """Emit rstd + LN application for a finished block. When
              use_rsqrt=False this must be emitted inside a Sqrt window."""
              mvh = pend['mvh']
              mv = mvh.rearrange("p (x k) -> p x k", k=2)
              veps = sm.tile([128, 8], F32, tag="veps", name="veps", bufs=2)
              ve = veps.rearrange("p (x k) -> p x k", k=1)
              nc.vector.tensor_scalar(ve[:], mv[:, :, 1:2], LN_EPS, None,
                                      op0=Alu.add)
              rstd = sm.tile([128, 8], F32, tag="rstd", name="rstd", bufs=2)
              vrec = sm.tile([128, 8], F32, tag="vrec", name="vrec", bufs=2)
              nc.vector.reciprocal(vrec[:], veps[:])
              nc.scalar.activation(rstd[:], vrec[:], Act.Sqrt)
              for pc in range(PC):
                  leng = nc.vector
                  leng.tensor_scalar(
                      pend['houts'][pc][:], pend['ybufs'][pc][:],
                      mvh[:, 2 * pc:2 * pc + 1], rstd[:, pc:pc + 1],
                      op0=Alu.subtract, op1=Alu.mult)

          # ---------- one attention block, triangular ----------
          def emit_block2(blk, QTp=None, V=None, E1t=None, post_c=None,
                          resid_dram=None, out_name="hout", pending=None):
              strict = (blk == 'b3')
              g2b = C['g2b_' + blk]
              E = {}
              bigA = {}

              # ---- phase A: E = exp(scores), lower-triangular chunks ----
              def qk(ps_view, b, h, c, Lc):
                  base = h * 1024 + b * 512
                  nc.tensor.matmul(
                      ps_view,
                      QTp[:, base + c * 128: base + c * 128 + 128],
                      QTp[:, base: base + Lc], start=True, stop=True)

              if not strict:
                  for b in range(BL):
                      for c in range(4):
                          Lc = _L(c)
                          Et = work.tile([128, 8 * Lc], BF16, tag=f"E{c}",
                                         bufs=2, name=f"E{blk}_{b}{c}")
                          E[(b, c)] = Et
                          for hp in range(4):
                              # ho-stride padded to 512 so each matmul's
                              # output stays inside one 2KB PSUM bank
                              ps = p1.tile([128, 1024], F32, tag="p1",
                                           name="qkps")
                              for ho in range(2):
                                  qk(ps[:, ho * 512: ho * 512 + Lc],
                                     b, 2 * hp + ho, c, Lc)
                              psv = ps.rearrange("p (t x) -> p t x",
                                                 x=512)[:, :, 0:Lc]
                              Ev = Et[:, 2 * hp * Lc:(2 * hp + 2) * Lc]\
                                  .rearrange("p (t x) -> p t x", x=Lc)
                              nc.scalar.activation(Ev, psv, Act.Exp)

              # ---- phase B: masked reverse scans + bigA ----
              # Scans write straight into the P{c} partner tiles (no Rv8):
              # head h occupies cols [h*Lc, (h+1)*Lc). The suffix slice for
              # head h reads one col past its range (the next head's scan
              # col 0, finite garbage); the chunk's last bigA column is
              # provably zero (suffix after the diagonal), so a per-head
              # memset fixes it after the fact.
              for b in range(BL):
                  for c in range(4):
                      Lc = _L(c)
                      Psc = work.tile([128, 8 * Lc + 1], BF16, tag=f"P{c}",
                                      bufs=1, name=f"scanP_{b}{c}")
                      nc.gpsimd.memset(Psc[:, 8 * Lc:8 * Lc + 1], 0.0)
                      mk = (C[f"mpad__{c}"][:, 1:Lc + 1] if strict
                            else C[f"mpad__{c}"][:, 0:Lc])
                      for h in range(8):
                          if strict:
                              grp, hh = divmod(h, 4)
                              e3 = pT.tile([128, 512], F32, tag="pT",
                                           name="e3p")
                              nc.tensor.matmul(
                                  e3[:, 0:Lc], ones1[0:1, 0:128],
                                  E1t[grp][0:1, hh * 1024 + b * 512:
                                           hh * 1024 + b * 512 + Lc],
                                  start=True, stop=True)
                              e3s = sm.tile([128, 512], BF16, tag="e3s",
                                            name="e3s", bufs=2)
                              rr_copy(e3s[:, 0:Lc], e3[:, 0:Lc])
                              src = e3s[:, 0:Lc]
                          else:
                              src = E[(b, c)][:, h * Lc:(h + 1) * Lc]
                          nc.vector.tensor_tensor_scan(
                              Psc[:, h * Lc: h * Lc + Lc][REV], src[REV],
                              mk[REV], 0.0, op0=Alu.add, op1=Alu.mult)
                      rv = Psc[:, 0:8 * Lc].rearrange("p (h x) -> p h x",
                                                      x=Lc)
                      r8m = sm.tile([128, 8], F32, tag="r8m", name="r8m",
                                    bufs=2)
                      r8v = r8m.rearrange("p (h x) -> p h x", x=1)
                      nc.vector.tensor_scalar(r8v[:], rv[:, :, 0:1], 1e-30,
                                              None, op0=Alu.max)
                      rc = sm.tile([128, 8], F32, tag="rc", name="rc", bufs=2)
                      nc.vector.reciprocal(rc[:], r8m[:])
                      rgb = sm.tile([128, 8], F32, tag="rgb", name="rgb",
                                    bufs=2)
                      nc.vector.tensor_tensor(rgb[:], rc[:], g2b[:],
                                              op=Alu.mult)
                      bA = work.tile([128, 8 * Lc], BF16, tag=f"bA{c}",
                                     bufs=2, name=f"bA_{b}{c}")
                      for h in range(8):
                          nc.vector.scalar_tensor_tensor(
                              bA[:, h * Lc:(h + 1) * Lc],
                              C[f"posm__{c}"][:, 0:Lc], rgb[:, h:h + 1],
                              Psc[:, h * Lc + 1: h * Lc + 1 + Lc],
                              op0=Alu.mult, op1=Alu.mult)
                          nc.gpsimd.memset(
                              bA[:, h * Lc + Lc - 1: h * Lc + Lc], 0.0)
                      bigA[(b, c)] = bA

              # ---- phase C: total = exp(-sqrt(bigA)) via partner
              # ping-pong (in-place ACT halves HW throughput), b-granular ----
              for b in range(BL):
                  Ps = {}
                  for c in range(4):
                      P = work.tile([128, 8 * _L(c)], BF16, tag=f"P{c}",
                                    bufs=1, name=f"P_{b}{c}")
                      nc.scalar.activation(P[:], bigA[(b, c)][:], Act.Sqrt)
                      Ps[c] = P
                  if b == 0 and pending is not None:
                      flush_ln(pending)      # Sqrt window: prev block's LN
                  for c in range(4):
                      nc.scalar.activation(bigA[(b, c)][:], Ps[c][:],
                                           Act.Exp, scale=-1.0)

              if post_c is not None:
                  V, wsb = post_c()

              # ---- phase D: z, Ez (in-place), AV, out-proj, LN stats ----
              houts = [work.tile([128, D], BF16, tag="hblk", bufs=8,
                                 name=f"{out_name}_{pc}") for pc in range(PC)]
              ybufs = {}
              mvh = sm.tile([128, 16], F32, tag="mvh", name="mvh", bufs=2)
              Wo = [C[blk + '_wo__0'], C[blk + '_wo__1']]
              obr = C[blk + '_obr']
              for b in range(BL):
                  for c in range(4):
                      Lc = _L(c)
                      bA = bigA[(b, c)]
                      if not strict:
                          # z = scores*total -> partner; Ez = exp(z) -> bigA
                          Pz = work.tile([128, 8 * Lc], BF16, tag=f"P{c}",
                                         bufs=1, name=f"Pz_{b}{c}")
                          for hp in range(4):
                              ps3 = p1.tile([128, 1024], F32, tag="p1",
                                            name="ps3")
                              for ho in range(2):
                                  qk(ps3[:, ho * 512: ho * 512 + Lc], b,
                                     2 * hp + ho, c, Lc)
                              ps3v = ps3.rearrange("p (t x) -> p t x",
                                                   x=512)[:, :, 0:Lc]
                              seg = slice(2 * hp * Lc, (2 * hp + 2) * Lc)
                              nc.vector.tensor_tensor(
                                  Pz[:, seg].rearrange("p (t x) -> p t x",
                                                       x=Lc),
                                  ps3v,
                                  bA[:, seg].rearrange("p (t x) -> p t x",
                                                       x=Lc),
                                  op=Alu.mult)
                          nc.scalar.activation(bA[:], Pz[:], Act.Exp)
                      else:
                          # transposed path: totalT chunks scaled by the
                          # score COLUMN (broadcast rows in the natural
                          # domain are columns here), then one batched exp.
                          # E{c} tags are unused by the strict block (no E
                          # tiles) and have matching slot sizes; reusing them
                          # avoids a WAR cycle against bA through the pT pool.
                          zT = work.tile([128, 8 * Lc], BF16, tag=f"E{c}",
                                         bufs=2, name="zT")
                          for h in range(8):
                              tp = pT.tile([128, 512], BF16, tag="pT",
                                           name="tpz")
                              for j in range(c + 1):
                                  nc.tensor.transpose(
                                      tp[:, j * 128:(j + 1) * 128],
                                      bA[:, h * Lc + j * 128:
                                         h * Lc + (j + 1) * 128], ident[:])
                              for j in range(c + 1):
                                  nc.vector.tensor_scalar(
                                      zT[:, h * Lc + j * 128:
                                         h * Lc + (j + 1) * 128],
                                      tp[:, j * 128:(j + 1) * 128],
                                      csbs[b * 4 + j][:, h:h + 1], None,
                                      op0=Alu.mult)
                          nc.scalar.activation(bA[:], zT[:], Act.Exp)
                          if c == 0:
                              for h in range(8):
                                  nc.gpsimd.memset(
                                      bA[:, h * Lc:h * Lc + 1], 0.0)

                      pav = pv.tile([128, 264], F32, tag="pv", name="pav")
                      for h in range(8):
                          if not strict:
                              tp = pT.tile([128, 512], BF16, tag="pT",
                                           name="tp")
                              for j in range(c + 1):
                                  nc.tensor.transpose(
                                      tp[:, j * 128:(j + 1) * 128],
                                      bA[:, h * Lc + j * 128:
                                         h * Lc + (j + 1) * 128], ident[:])
                              ats = sm.tile([128, 512], BF16, tag="ATs",
                                            bufs=2, name="ats")
                              rr_copy(ats[:, 0:(c + 1) * 128],
                                      tp[:, 0:(c + 1) * 128])
                          seg = pav[:, h * 33:(h + 1) * 33]
                          for jc in range(4):
                              st = (jc == 0)
                              sp = (jc == 3)
                              if jc <= c:
                                  lhsT = (bA[:, h * Lc + jc * 128:
                                             h * Lc + (jc + 1) * 128]
                                          if strict else
                                          ats[:, jc * 128:(jc + 1) * 128])
                                  nc.tensor.matmul(
                                      seg, lhsT,
                                      V[b * 4 + jc][:, h * 33:(h + 1) * 33],
                                      start=st, stop=sp)
                              elif strict:
                                  nc.tensor.matmul(
                                      seg, ones1[0:1, 0:128],
                                      wsb[b][0:1, (jc - 1) * 264 + h * 33:
                                             (jc - 1) * 264 + h * 33 + 33],
                                      start=st, stop=sp)
                              else:
                                  Lj = _L(jc)
                                  nc.tensor.matmul(
                                      seg,
                                      E[(b, jc)][:, h * Lj + c * 128:
                                                 h * Lj + (c + 1) * 128],
                                      V[b * 4 + jc][:, h * 33:(h + 1) * 33],
                                      start=st, stop=sp)

                      pav3 = pav.rearrange("p (h c) -> p h c", c=33)
                      rsm = sm.tile([128, H], F32, tag="rsm", name="rsm")
                      nc.vector.tensor_scalar(rsm[:], pav3[:, :, 32:33],
                                              1e-30, None, op0=Alu.max)
                      rsi = sm.tile([128, H], F32, tag="rsi", name="rsi")
                      nc.vector.reciprocal(rsi[:], rsm[:])
                      att_sb = sm.tile([128, 256], BF16, tag="att",
                                       name="att", bufs=2)
                      nc.scalar.activation(
                          att_sb.rearrange("p (h c) -> p h c", c=32),
                          pav3[:, :, 0:32], Act.Copy)
                      if strict and c == 0:
                          nc.gpsimd.memset(att_sb[0:1, :], 0.0)
                      att_n = sm.tile([128, 256], BF16, tag="attn",
                                      name="attn", bufs=2)
                      for h in range(H):
                          aeng = nc.vector
                          aeng.tensor_scalar(
                              att_n[:, h * 32:(h + 1) * 32],
                              att_sb[:, h * 32:(h + 1) * 32],
                              rsi[:, h:h + 1], None, op0=Alu.mult)
                      attT = [sm.tile([128, 128], BF16, tag=f"attT{i}",
                                      name="attT", bufs=2)
                              for i in range(2)]
                      for dc in range(2):
                          transpose128(attT[dc][:],
                                       att_n[:, dc * 128:(dc + 1) * 128])
                      po = pv.tile([128, 264], F32, tag="pv", name="po")
                      nc.tensor.matmul(po[:, 0:256], attT[0][:], Wo[0][:],
                                       start=True, stop=False)
                      nc.tensor.matmul(po[:, 0:256], attT[1][:], Wo[1][:],
                                       start=False, stop=False)
                      nc.tensor.matmul(po[:, 0:256], ones1[0:1, 0:128],
                                       obr[:], start=False, stop=True)
                      y = sm.tile([128, D], BF16, tag="ybuf", name="y",
                                  bufs=8)
                      if resid_dram is not None:
                          resid = sm.tile([128, D], F32, tag="xin",
                                          name="resid", bufs=2)
                          nc.sync.dma_start(
                              resid[:],
                              resid_dram[b, c * 128:(c + 1) * 128, :])
                          nc.vector.tensor_tensor(y[:], po[:, 0:256],
                                                  resid[:], op=Alu.add)
                      else:
                          nc.vector.tensor_copy(y[:], po[:, 0:256])
                      pc = b * 4 + c
                      ybufs[pc] = y
                      st6 = sm.tile([128, 6], F32, tag="st6", name="st6")
                      nc.vector.bn_stats(st6[:], y[:])
                      nc.vector.bn_aggr(mvh[:, 2 * pc:2 * pc + 2], st6[:])
              return dict(mvh=mvh, ybufs=ybufs, houts=houts)

          def transpose_chunks(chunks, out_name):
              out = [work.tile([128, BL * S], BF16, tag="xfrm", bufs=4,
                               name=f"{out_name}_{dc}") for dc in range(2)]
              for pc in range(PC):
                  for dc in range(2):
                      transpose128(out[dc][:, pc * 128:(pc + 1) * 128],
                                   chunks[pc][:, dc * 128:(dc + 1) * 128])
              return out

          # ================= blocks 1, 2 =================
          xT1 = [xT[(1, 0)], xT[(1, 1)]]
          xT2 = [xT[(2, 0)], xT[(2, 1)]]
          QT1 = projQ(xT1, 'b1_wq', 'b1_qbr', 'QT1')
          V1 = projN(xT1, 'b1_wv', 'b1_vbr', 'V1')
          pend1 = emit_block2('b1', QTp=QT1, V=V1, resid_dram=x1d,
                              out_name='hq')
          QT2 = projQ(xT2, 'b2_wq', 'b2_qbr', 'QT2')
          V2 = projN(xT2, 'b2_wv', 'b2_vbr', 'V2')
          pend2 = emit_block2('b2', QTp=QT2, V=V2, resid_dram=x2d,
                              out_name='ha', pending=pend1)

          # ================= block 3 setup =================
          # hq ready after pend1 flush (inside b2's Sqrt window)
          hqT = transpose_chunks(pend1['houts'], 'hqT')
          K3T = projT(hqT, 'b3_wk', 'b3_kbT', 'K3T')
          # Block-3 scores are broadcast rows; keep them as per-chunk COLUMNS
          # csbs[pc][128, H] (z applied in the transposed domain), plus
          # E1t[grp] [1, 4096] row tiles (scan-input broadcast rhs) and
          # E1d8[b] [8, 512] (rank-1 tail w vectors).
          E1t = [work.tile([1, 4 * 1024], BF16, tag="row4k", bufs=2,
                           name=f"e1t_{grp}") for grp in range(2)]
          E1d8 = [work.tile([8, 512], BF16, tag="e1d8", bufs=2,
                            name=f"e1d8_{b}") for b in range(BL)]
          csbs = []
          for pc in range(PC):
              b, ic = divmod(pc, 4)
              psc = pv.tile([128, 264], F32, tag="pv", name="psc")
              for kc in range(2):
                  nc.tensor.matmul(psc[:, 0:H],
                                   K3T[kc][:, pc * 128:(pc + 1) * 128],
                                   C[f"q03__{kc}"][:],
                                   start=(kc == 0), stop=(kc == 1))
              csb = sm.tile([128, H], F32, tag="csb", name="csb", bufs=8)
              nc.vector.tensor_copy(csb[:], psc[:, 0:H])
              csbs.append(csb)
              csbE = sm.tile([128, H], BF16, tag="csbE", name="csbE", bufs=2)
              nc.scalar.activation(csbE[:], psc[:, 0:H], Act.Exp)
              for grp in range(2):
                  ptE = pT.tile([128, 512], BF16, tag="pT", name="ptE")
                  for hh in range(4):
                      h = grp * 4 + hh
                      nc.tensor.transpose(ptE[0:1, hh * 128:(hh + 1) * 128],
                                          csbE[:, h:h + 1], ident[:])
                  srcE = ptE[0:1, 0:512].rearrange("p (h c) -> p h c", h=4)
                  dviewE = E1t[grp].rearrange("p (h c) -> p h c", h=4)[
                      0:1, :, pc * 128:pc * 128 + 128]
                  nc.vector.tensor_copy(dviewE, srcE)
              # E1d8: transpose csbE [128 rows=i, 8 cols=h] -> [8, 128]
              pt8 = pT.tile([128, 512], BF16, tag="pT", name="pt8")
              nc.tensor.transpose(pt8[0:8, 0:128], csbE[:, 0:8], ident[:])
              nc.vector.tensor_copy(E1d8[b][:, ic * 128:(ic + 1) * 128],
                                    pt8[0:8, 0:128])

          # E1dT_sb [128, 64]: cols (b*4+jc)*8 + h = E1d8[b][h, jc*128+k]
          E1dT_sb = work.tile([128, 64], BF16, tag="e1dT", bufs=1,
                              name="e1dT")
          for b in range(BL):
              for jc in range(4):
                  ptd = pT.tile([128, 512], BF16, tag="pT", name="ptd")
                  nc.tensor.transpose(ptd[0:128, 0:8],
                                      E1d8[b][:, jc * 128:(jc + 1) * 128],
                                      ident[0:8, 0:8])
                  nc.vector.tensor_copy(
                      E1dT_sb[:, (b * 4 + jc) * 8:(b * 4 + jc) * 8 + 8],
                      ptd[0:128, 0:8])

          def post_c_b3():
              """After b3's Sqrt window (which flushed pend2): ha ready."""
              haT = transpose_chunks(pend2['houts'], 'haT')
              V3 = projN(haT, 'b3_wv', 'b3_vbr', 'V3')
              # rank-1 tails: w(b,jc)[1, 264] = E1d(chunk jc) . V3[b*4+jc]
              wsb = [work.tile([1, 3 * 264], BF16, tag="wsb", bufs=2,
                               name=f"wsb_{b}") for b in range(BL)]
              for b in range(BL):
                  for jc in range(1, 4):
                      wp = pv.tile([128, 264], F32, tag="pv", name="wp")
                      for h in range(H):
                          nc.tensor.matmul(
                              wp[0:1, h * 33:(h + 1) * 33],
                              E1dT_sb[:, (b * 4 + jc) * 8 + h:
                                      (b * 4 + jc) * 8 + h + 1],
                              V3[b * 4 + jc][:, h * 33:(h + 1) * 33],
                              start=True, stop=True)
                      nc.vector.tensor_copy(
                          wsb[b][0:1, (jc - 1) * 264:jc * 264], wp[0:1, :])
              return V3, wsb

          pend3 = emit_block2('b3', E1t=E1t, post_c=post_c_b3,
                              resid_dram=None, out_name='h3', pending=pend2)
          flush_ln(pend3, use_rsqrt=True)
          h3 = pend3['houts']

          # ================= final stage =================
          for pc in range(PC):
              b, ic = divmod(pc, 4)
              # per-pc transposed h3: h3T [32, 8*128] cols h*128 + i
              h3T = sm.tile([32, 1024], BF16, tag="h3T", bufs=2, name="h3T")
              for grp in range(2):
                  ptv = pT.tile([128, 512], BF16, tag="pT", name="ptv")
                  for hh in range(4):
                      h = grp * 4 + hh
                      nc.tensor.transpose(ptv[0:32, hh * 128:(hh + 1) * 128],
                                          h3[pc][:, h * 32:(h + 1) * 32],
                                          ident[:])
                  rr_copy(h3T[:, grp * 512:(grp + 1) * 512], ptv[0:32, :])
              vhalves = []
              for half in range(2):
                  ps = p1.tile([128, 1024], F32, tag="p1", name="val_ps")
                  for hh in range(4):
                      h = half * 4 + hh
                      seg = ps[:, hh * 256:(hh + 1) * 256]
                      nc.tensor.matmul(seg, h3T[0:32, h * 128:(h + 1) * 128],
                                       C[f"lvw__{h}"][:],
                                       start=True, stop=False)
                      nc.tensor.matmul(seg, ones1[0:1, 0:128],
                                       C['lvbr'][0:1, h * 256:(h + 1) * 256],
                                       start=False, stop=True)
                  val = sm.tile([128, 1024], BF16, tag="val", bufs=2,
                                name="val")
                  nc.scalar.activation(val[:], ps[:], Act.Sigmoid)
                  vhalves.append(val)
              alpha = alphas[pc]
              acc = sm.tile([128, D], F32, tag="acc", name="acc", bufs=2)
              nc.vector.tensor_scalar(
                  acc[:], vhalves[0][:, 0:256],
                  alpha[:, 0:1], None, op0=Alu.mult)
              for h in range(1, H):
                  half, hh = divmod(h, 4)
                  acc2 = sm.tile([128, D], F32, tag="acc", name="acc2",
                                 bufs=2)
                  nc.vector.scalar_tensor_tensor(
                      acc2[:], vhalves[half][:, hh * 256:(hh + 1) * 256],
                      alpha[:, h:h + 1], acc[:],
                      op0=Alu.mult, op1=Alu.add)
                  acc = acc2
              nc.sync.dma_start(outd[b, ic * 128:(ic + 1) * 128, :], acc[:])

    nc.compile()
    return nc


_GRAPH_CACHE = {}


def _get_graph(consts, g2):
    key = tuple(np.float32(v) for blk in ('b1', 'b2', 'b3')
                for v in g2[blk])
    if key not in _GRAPH_CACHE:
        _GRAPH_CACHE[key] = _build(consts, g2)
    return _GRAPH_CACHE[key]


def kernel(**inputs):
    consts, g2 = _host_prep(inputs)
    nc = _get_graph(consts, g2)
    q = np.ascontiguousarray(np.asarray(inputs['q_emb'], np.float32))
    qa = np.ascontiguousarray(np.asarray(inputs['qa_emb'], np.float32))
    in_maps = []
    for core in range(NCORES):
        m = {'x1': q[core * BL:(core + 1) * BL],
             'x2': qa[core * BL:(core + 1) * BL]}
        m.update(consts)
        in_maps.append(m)
    res = run_bass_kernel_spmd(nc, in_maps, core_ids=list(range(NCORES)))
    out = np.concatenate([res.results[c]['out'] for c in range(NCORES)],
                         axis=0)
    return out.astype(np.float32)
